# revision 16
# baseline (speedup 1.0000x reference)
"""Trainium2 Bass kernel for nn_GTN_72679436583060 (GTN message passing).

Math: with w-softmax over a singleton axis each GTConv is exactly 2*A, so

    out = 2 * rownorm(4*A@A + I) @ A
        = diag(64 / (16*rowsum(M1) + 1)) @ (M1@Ah + 0.0625*Ah_rows)

with Ah = A/2 (so M1 = Ah@Ah = (A@A)/4 ~ 128 fits fp8e4m3 range) and
rowsum/deg folded into a per-row reciprocal scale.

Sharding: row-wise over 8 cores, Ah replicated. Per core (rows R = 256):
  GEMM1 (transposed):  MT = Ah^T @ (Ah_rows^T)      (2048 x 256)
  deg:                 rowsum(M1) via ones-column matmuls on MT
  GEMM2:               P = M1 @ Ah + 0.0625*Ah_rows  (256 x 2048)
  epilogue:            out = P * (64 / (16*deg1 + 1)) per-row, bf16 out

All matmuls in fp8e4m3 with perf_mode=DoubleRow (K=256 per instruction,
two k-slabs per 3D access pattern [128, 2, f]), fp32 PSUM accumulation.
GEMM1 runs k-pair-outer so the PE tracks the streaming A DMA; warmup
matmuls during the initial DMA window ramp the PE HAM clock. Output is
DMA'd as bf16 and upcast on the host to halve the output tail.
"""

import numpy as np

N = 2048
P = 128
NCORES = 8
R = N // NCORES        # 256 rows per core
KT = N // P            # 16 partition tiles
KP = KT // 2           # 8 k-pair tiles (DoubleRow)
MT = R // P            # 2 row subtiles per core
FD = 512               # PSUM bank free dim (fp32)
NT2 = N // FD          # 4 GEMM2 n-tiles

_CACHE = {}


def _build_bass():
    from contextlib import ExitStack

    import concourse.bass as bass  # noqa: F401
    import concourse.mybir as mybir
    import concourse.tile as tile
    from concourse import bacc

    dt = mybir.dt
    fp32 = dt.float32
    bf16 = dt.bfloat16
    fp8 = dt.float8e4
    Alu = mybir.AluOpType
    DR = mybir.MatmulPerfMode.DoubleRow

    nc = bacc.Bacc(None, target_bir_lowering=False)
    # a/ar are k-slab shuffled on the host: [p, k, :] = X[k*128 + p, :],
    # so one DMA ring moves a whole 2-slab pair (4KB contiguous/partition).
    a_d = nc.dram_tensor("a", [P, KT, N], fp8, kind="ExternalInput")
    art_d = nc.dram_tensor("art", [P, KT, R], fp8, kind="ExternalInput")
    ar_d = nc.dram_tensor("ar", [P, 2, N], fp8, kind="ExternalInput")
    ones_d = nc.dram_tensor("ones", [P, 1], fp8, kind="ExternalInput")
    iq_d = nc.dram_tensor("iq", [P, P], fp8, kind="ExternalInput")
    out_d = nc.dram_tensor("out", [R, N], bf16, kind="ExternalOutput")

    with tile.TileContext(nc) as tc, ExitStack() as ctx:
        a_pool = ctx.enter_context(tc.tile_pool(name="a", bufs=KP))
        art_pool = ctx.enter_context(tc.tile_pool(name="art", bufs=KP))
        ar_pool = ctx.enter_context(tc.tile_pool(name="ar", bufs=1))
        mt_pool = ctx.enter_context(tc.tile_pool(name="mt", bufs=KP))
        const_pool = ctx.enter_context(tc.tile_pool(name="const", bufs=1))
        outsb_pool = ctx.enter_context(tc.tile_pool(name="outsb", bufs=4))
        sc_pool = ctx.enter_context(tc.tile_pool(name="sc", bufs=4))

        zeros_t = const_pool.tile([P, FD], bf16, tag="zeros")
        nc.vector.memset(zeros_t[:], 0.0)

        # Stream A k-slab pairs in k order; they stay resident: GEMM1 uses
        # A slabs as lhsT, GEMM2 reuses them as rhs. Each HWDGE queue
        # sustains only ~150 GB/s, so the stream is spread over all three
        # DMA-capable engines (sync/SP, scalar/ACT, gpsimd). The first two
        # pairs are split across queues so GEMM1 starts early; the small
        # art pairs ride the gpsimd queue between big rings, always one
        # step ahead of the matching a pair. ar/iq/ones (GEMM2-only) go
        # last.
        art_tiles = [
            art_pool.tile([P, 2, R], fp8, tag="art", name=f"art_{t}")
            for t in range(KP)
        ]
        a_tiles = [
            a_pool.tile([P, 2, N], fp8, tag="a", name=f"a_{t}")
            for t in range(KP)
        ]

        def art_ring(t):
            nc.gpsimd.dma_start(art_tiles[t][:], art_d[:, 2 * t:2 * t + 2, :])

        def a_ring(eng, t):
            eng.dma_start(a_tiles[t][:], a_d[:, 2 * t:2 * t + 2, :])

        art_ring(0)
        nc.sync.dma_start(a_tiles[0][:, 0, :], a_d[:, 0, :])
        nc.scalar.dma_start(a_tiles[0][:, 1, :], a_d[:, 1, :])
        art_ring(1)
        nc.gpsimd.dma_start(a_tiles[1][:, 0, :], a_d[:, 2, :])
        nc.scalar.dma_start(a_tiles[1][:, 1, :], a_d[:, 3, :])
        art_ring(2)
        art_ring(3)
        a_ring(nc.sync, 2)
        a_ring(nc.scalar, 3)
        nc.gpsimd.dma_start(a_tiles[4][:, 0, :], a_d[:, 8, :])
        nc.sync.dma_start(a_tiles[4][:, 1, :], a_d[:, 9, :])
        art_ring(4)
        art_ring(5)
        a_ring(nc.sync, 5)
        a_ring(nc.scalar, 6)
        art_ring(6)
        art_ring(7)
        a_ring(nc.gpsimd, 7)
        ar_t = ar_pool.tile([P, 2, N], fp8, tag="ar")
        nc.gpsimd.dma_start(ar_t[:], ar_d[:, :, :])
        ones_t = const_pool.tile([P, 1], fp8, tag="ones")
        nc.gpsimd.dma_start(ones_t[:], ones_d[:, :])
        iq_t = const_pool.tile([P, P], fp8, tag="iq")
        nc.gpsimd.dma_start(iq_t[:], iq_d[:, :])

        # ---- GEMM1: MT[j, r] = sum_k Ah[k, j] * Ah_rows[r, k] ----
        # k-pair outer (DoubleRow contracts 256 rows per matmul). Two
        # j-groups share each PSUM bank: the even j's first matmul carries
        # start=True (resets has_written for the whole bank), the odd j's
        # first matmul then overwrites its all-unwritten half; later
        # matmuls accumulate. The PE executes its queue in program order,
        # so the start=True matmul always lands first. Warmup matmuls on
        # bank 7 keep the PE busy during the initial DMA window so the HAM
        # clock ramps (bank 7's real start=True matmul discards them).
        with tc.tile_pool(name="psum", bufs=8, space="PSUM") as psum_pool:
            pairs = []
            for b in range(KP):
                ps = psum_pool.tile([P, FD], fp32, tag="bank", name=f"pair_{b}")
                pairs.append(ps)
            for w in range(3):
                nc.tensor.matmul(
                    pairs[7][:], zeros_t[:, 0:P], zeros_t[:],
                    start=(w == 0), stop=False, skip_group_check=True,
                )
            for t in range(KP):
                for j in range(KT):
                    half = (j % 2) * R
                    nc.tensor.matmul(
                        pairs[j // 2][:, half:half + R],
                        a_tiles[t][:, :, j * P:(j + 1) * P],
                        art_tiles[t][:],
                        start=(t == 0 and j % 2 == 0), stop=(t == KP - 1),
                        perf_mode=DR, skip_group_check=True,
                    )
            # PSUM -> SBUF fp8 quantization of MT (values ~128 < 240 max),
            # split across the vector (DVE) and scalar (ACT) engines —
            # gpsimd cannot access PSUM.
            mt_tiles = [None] * KP
            for j in range(KT):
                half = (j % 2) * R
                if j % 2 == 0:
                    mt_tiles[j // 2] = mt_pool.tile(
                        [P, 2, R], fp8, tag="mt", name=f"mt_{j // 2}"
                    )
                dst = mt_tiles[j // 2][:, j % 2, :]
                src = pairs[j // 2][:, half:half + R]
                if j % 2 == 0:
                    nc.vector.tensor_copy(dst, src)
                else:
                    nc.scalar.copy(dst, src)

            # ---- GEMM2 + deg + epilogue ----
            # The 0.0625*I seed matmul doubles as each bank's accumulation
            # starter (start=True clears the bank and seeds 0.0625*Ah_rows).
            # m=0 runs jp-outer (tracks the mt copies); m=1 runs n-outer so
            # its banks finish staggered and the epilogues pipeline with PE.
            def emit_epilogue(m, n, psum_tile, sca, last=False):
                ot = outsb_pool.tile([P, FD], bf16, tag="ot",
                                     name=f"ot_{m}_{n}")
                if last:
                    # Final tile is on the critical tail: split the scale
                    # across DVE+ACT and the writeback across two queues.
                    nc.vector.tensor_scalar(
                        out=ot[:, 0:R], in0=psum_tile[:, 0:R],
                        scalar1=sca[:], scalar2=None, op0=Alu.mult,
                    )
                    nc.scalar.mul(ot[:, R:FD], psum_tile[:, R:FD], sca[:])
                    base = n * FD
                    nc.sync.dma_start(
                        out_d[m * P:(m + 1) * P, base:base + R], ot[:, 0:R]
                    )
                    nc.gpsimd.dma_start(
                        out_d[m * P:(m + 1) * P, base + R:base + FD],
                        ot[:, R:FD],
                    )
                    return
                if n % 2 == 0:
                    nc.vector.tensor_scalar(
                        out=ot[:], in0=psum_tile[:], scalar1=sca[:],
                        scalar2=None, op0=Alu.mult,
                    )
                else:
                    nc.scalar.mul(ot[:], psum_tile[:], sca[:])
                eng = (nc.sync, nc.gpsimd, nc.scalar)[n % 3]
                eng.dma_start(
                    out_d[m * P:(m + 1) * P, n * FD:(n + 1) * FD], ot[:]
                )

            def emit_deg_scale(m, deg_ps):
                # scale = 64 / (16*deg1 + 1) == 1 / (0.25*deg1 + 0.015625)
                t1 = sc_pool.tile([P, 1], fp32, tag="t1", name=f"t1_{m}")
                nc.vector.tensor_scalar(
                    out=t1[:], in0=deg_ps[:], scalar1=0.25, scalar2=0.015625,
                    op0=Alu.mult, op1=Alu.add,
                )
                sca = sc_pool.tile([P, 1], fp32, tag="sca", name=f"sca_{m}")
                nc.vector.reciprocal(sca[:], t1[:])
                return sca

            # m = 0: jp-outer
            m = 0
            outs_ps = [psum_pool.tile([P, FD], fp32, tag="bank",
                                      name=f"outps0_{i}") for i in range(NT2)]
            deg_full = psum_pool.tile([P, FD], fp32, tag="bank", name="deg_0")
            deg_ps = deg_full[:, 0:1]
            for n in range(NT2):
                nc.tensor.matmul(
                    outs_ps[n][:], iq_t[:],
                    ar_t[:, m, n * FD:(n + 1) * FD],
                    start=True, stop=False, skip_group_check=True,
                )
            for t in range(KP):
                lhsT = mt_tiles[t][:, :, m * P:(m + 1) * P]
                for n in range(NT2):
                    nc.tensor.matmul(
                        outs_ps[n][:], lhsT,
                        a_tiles[t][:, :, n * FD:(n + 1) * FD],
                        start=False, stop=(t == KP - 1),
                        perf_mode=DR, skip_group_check=True,
                    )
                for i in range(2):
                    nc.tensor.matmul(
                        deg_ps[:], mt_tiles[t][:, i, m * P:(m + 1) * P],
                        ones_t[:],
                        start=(t == 0 and i == 0), stop=(t == KP - 1 and i == 1),
                    )
            sca = emit_deg_scale(m, deg_ps)
            for n in range(NT2):
                emit_epilogue(m, n, outs_ps[n], sca)

            # m = 1: n-outer, deg rides along with the n=0 bank
            m = 1
            deg_full = psum_pool.tile([P, FD], fp32, tag="bank", name="deg_1")
            deg_ps = deg_full[:, 0:1]
            sca = None
            for n in range(NT2):
                ops = psum_pool.tile([P, FD], fp32, tag="bank",
                                     name=f"outps1_{n}")
                nc.tensor.matmul(
                    ops[:], iq_t[:], ar_t[:, m, n * FD:(n + 1) * FD],
                    start=True, stop=False, skip_group_check=True,
                )
                for t in range(KP):
                    nc.tensor.matmul(
                        ops[:], mt_tiles[t][:, :, m * P:(m + 1) * P],
                        a_tiles[t][:, :, n * FD:(n + 1) * FD],
                        start=False, stop=(t == KP - 1),
                        perf_mode=DR, skip_group_check=True,
                    )
                    if n == 0:
                        for i in range(2):
                            nc.tensor.matmul(
                                deg_ps[:],
                                mt_tiles[t][:, i, m * P:(m + 1) * P],
                                ones_t[:],
                                start=(t == 0 and i == 0),
                                stop=(t == KP - 1 and i == 1),
                            )
                if n == 0:
                    sca = emit_deg_scale(m, deg_ps)
                emit_epilogue(m, n, ops, sca, last=(n == NT2 - 1))
    nc.compile()
    return nc


def _get_nc():
    if "nc" not in _CACHE:
        _CACHE["nc"] = _build_bass()
    return _CACHE["nc"]


def _make_in_maps(A_f32):
    import ml_dtypes

    f8 = ml_dtypes.float8_e4m3
    Ah = (A_f32 * 0.5).astype(f8)
    ATh = np.ascontiguousarray(Ah.T)
    # k-slab shuffle: a_shuf[p, k, :] = Ah[k*128 + p, :]
    a_shuf = np.ascontiguousarray(Ah.reshape(KT, P, N).transpose(1, 0, 2))

    ones = np.ones((P, 1), dtype=f8)
    iq = (0.0625 * np.eye(P, dtype=np.float32)).astype(f8)
    in_maps = []
    for c in range(NCORES):
        sl = slice(c * R, (c + 1) * R)
        art = np.ascontiguousarray(
            ATh[:, sl].reshape(KT, P, R).transpose(1, 0, 2)
        )
        ar = np.ascontiguousarray(
            Ah[sl, :].reshape(2, P, N).transpose(1, 0, 2)
        )
        in_maps.append({
            "a": a_shuf,
            "art": art,
            "ar": ar,
            "ones": ones,
            "iq": iq,
        })
    return in_maps


def kernel(A, w1a=None, w1b=None, w2a=None, **_unused):
    # w1a/w1b/w2a only enter the reference through a softmax over a
    # singleton axis (== 1.0), so the output does not depend on them.
    from concourse.bass_utils import run_bass_kernel_spmd

    A = np.asarray(A, dtype=np.float32)
    assert A.shape == (N, N), A.shape
    nc = _get_nc()
    in_maps = _make_in_maps(A)
    res = run_bass_kernel_spmd(nc, in_maps, core_ids=list(range(NCORES)))
    out = np.concatenate(
        [res.results[c]["out"] for c in range(NCORES)], axis=0
    )
    return out[None].astype(np.float32)


# revision 17
# speedup vs baseline: 1.0158x; 1.0158x over previous
"""Trainium2 Bass kernel for nn_GTN_72679436583060 (GTN message passing).

Math: with w-softmax over a singleton axis each GTConv is exactly 2*A, so

    out = 2 * rownorm(4*A@A + I) @ A
        = diag(64 / (16*rowsum(M1) + 1)) @ (M1@Ah + 0.0625*Ah_rows)

with Ah = A/2 (so M1 = Ah@Ah = (A@A)/4 ~ 128 fits fp8e4m3 range) and
rowsum/deg folded into a per-row reciprocal scale.

Sharding: row-wise over 8 cores, Ah replicated. Per core (rows R = 256):
  GEMM1 (transposed):  MT = Ah^T @ (Ah_rows^T)      (2048 x 256)
  deg:                 rowsum(M1) via ones-column matmuls on MT
  GEMM2:               P = M1 @ Ah + 0.0625*Ah_rows  (256 x 2048)
  epilogue:            out = P * (64 / (16*deg1 + 1)) per-row, bf16 out

All matmuls in fp8e4m3 with perf_mode=DoubleRow (K=256 per instruction,
two k-slabs per 3D access pattern [128, 2, f]), fp32 PSUM accumulation.
GEMM1 runs k-pair-outer so the PE tracks the streaming A DMA. Each HWDGE
queue sustains only ~150 GB/s, so the A stream is split slab-wise over
the sync+scalar queues while all small transfers (art/ar/iq/ones) ride
the gpsimd queue. Output is DMA'd as bf16 and upcast on the host.
"""

import numpy as np

N = 2048
P = 128
NCORES = 8
R = N // NCORES        # 256 rows per core
KT = N // P            # 16 partition tiles
KP = KT // 2           # 8 k-pair tiles (DoubleRow)
MT = R // P            # 2 row subtiles per core
FD = 512               # PSUM bank free dim (fp32)
NT2 = N // FD          # 4 GEMM2 n-tiles

_CACHE = {}


def _build_bass():
    from contextlib import ExitStack

    import concourse.bass as bass  # noqa: F401
    import concourse.mybir as mybir
    import concourse.tile as tile
    from concourse import bacc

    dt = mybir.dt
    fp32 = dt.float32
    bf16 = dt.bfloat16
    fp8 = dt.float8e4
    Alu = mybir.AluOpType
    DR = mybir.MatmulPerfMode.DoubleRow

    nc = bacc.Bacc(None, target_bir_lowering=False)
    # a/ar are k-slab shuffled on the host: [p, k, :] = X[k*128 + p, :].
    a_d = nc.dram_tensor("a", [P, KT, N], fp8, kind="ExternalInput")
    art_d = nc.dram_tensor("art", [P, KT, R], fp8, kind="ExternalInput")
    ar_d = nc.dram_tensor("ar", [P, 2, N], fp8, kind="ExternalInput")
    ones_d = nc.dram_tensor("ones", [P, 1], fp8, kind="ExternalInput")
    iq_d = nc.dram_tensor("iq", [P, P], fp8, kind="ExternalInput")
    out_d = nc.dram_tensor("out", [R, N], bf16, kind="ExternalOutput")

    with tile.TileContext(nc) as tc, ExitStack() as ctx:
        a_pool = ctx.enter_context(tc.tile_pool(name="a", bufs=KP))
        art_pool = ctx.enter_context(tc.tile_pool(name="art", bufs=KP))
        ar_pool = ctx.enter_context(tc.tile_pool(name="ar", bufs=1))
        mt_pool = ctx.enter_context(tc.tile_pool(name="mt", bufs=KP))
        const_pool = ctx.enter_context(tc.tile_pool(name="const", bufs=1))
        outsb_pool = ctx.enter_context(tc.tile_pool(name="outsb", bufs=4))
        sc_pool = ctx.enter_context(tc.tile_pool(name="sc", bufs=4))

        zeros_t = const_pool.tile([P, FD], bf16, tag="zeros")
        nc.vector.memset(zeros_t[:], 0.0)

        # Stream A k-slab pairs in k order; they stay resident: GEMM1 uses
        # A slabs as lhsT, GEMM2 reuses them as rhs. Slab 0 of each pair
        # goes on the sync queue, slab 1 on the scalar queue (so a pair
        # lands in ~1.7us); art0 leads the scalar queue so GEMM1's first
        # matmul isn't blocked on the later-starting gpsimd queue, which
        # carries the remaining art pairs and the GEMM2-only ar/ones/iq.
        art_tiles = [
            art_pool.tile([P, 2, R], fp8, tag="art", name=f"art_{t}")
            for t in range(KP)
        ]
        a_tiles = [
            a_pool.tile([P, 2, N], fp8, tag="a", name=f"a_{t}")
            for t in range(KP)
        ]
        nc.scalar.dma_start(art_tiles[0][:], art_d[:, 0:2, :])
        for t in range(KP):
            nc.sync.dma_start(a_tiles[t][:, 0, :], a_d[:, 2 * t, :])
            nc.scalar.dma_start(a_tiles[t][:, 1, :], a_d[:, 2 * t + 1, :])
        for t in range(1, KP):
            nc.gpsimd.dma_start(
                art_tiles[t][:], art_d[:, 2 * t:2 * t + 2, :]
            )
        ar_t = ar_pool.tile([P, 2, N], fp8, tag="ar")
        nc.gpsimd.dma_start(ar_t[:], ar_d[:, :, :])
        ones_t = const_pool.tile([P, 1], fp8, tag="ones")
        nc.gpsimd.dma_start(ones_t[:], ones_d[:, :])
        iq_t = const_pool.tile([P, P], fp8, tag="iq")
        nc.gpsimd.dma_start(iq_t[:], iq_d[:, :])

        # ---- GEMM1: MT[j, r] = sum_k Ah[k, j] * Ah_rows[r, k] ----
        # k-pair outer (DoubleRow contracts 256 rows per matmul). Two
        # j-groups share each PSUM bank: the even j's first matmul carries
        # start=True (resets has_written for the whole bank), the odd j's
        # first matmul then overwrites its all-unwritten half; later
        # matmuls accumulate. The PE executes its queue in program order,
        # so the start=True matmul always lands first. Warmup matmuls on
        # bank 7 keep the PE busy during the initial DMA window so the HAM
        # clock ramps (bank 7's real start=True matmul discards them).
        with tc.tile_pool(name="psum", bufs=8, space="PSUM") as psum_pool:
            pairs = []
            for b in range(KP):
                ps = psum_pool.tile([P, FD], fp32, tag="bank", name=f"pair_{b}")
                pairs.append(ps)
            for w in range(3):
                nc.tensor.matmul(
                    pairs[7][:], zeros_t[:, 0:P], zeros_t[:],
                    start=(w == 0), stop=False, skip_group_check=True,
                )
            for t in range(KP):
                for j in range(KT):
                    half = (j % 2) * R
                    nc.tensor.matmul(
                        pairs[j // 2][:, half:half + R],
                        a_tiles[t][:, :, j * P:(j + 1) * P],
                        art_tiles[t][:],
                        start=(t == 0 and j % 2 == 0), stop=(t == KP - 1),
                        perf_mode=DR, skip_group_check=True,
                    )
            # PSUM -> SBUF fp8 quantization of MT (values ~128 < 240 max)
            mt_tiles = [None] * KP
            for j in range(KT):
                half = (j % 2) * R
                if j % 2 == 0:
                    mt_tiles[j // 2] = mt_pool.tile(
                        [P, 2, R], fp8, tag="mt", name=f"mt_{j // 2}"
                    )
                nc.vector.tensor_copy(
                    mt_tiles[j // 2][:, j % 2, :],
                    pairs[j // 2][:, half:half + R],
                )

            # ---- GEMM2 + deg + epilogue ----
            # The 0.0625*I seed matmul doubles as each bank's accumulation
            # starter (start=True clears the bank and seeds 0.0625*Ah_rows).
            # m=0 runs jp-outer (tracks the mt copies); m=1 runs n-outer so
            # its banks finish staggered and the epilogues pipeline with PE.
            def emit_epilogue(m, n, psum_tile, sca, last=False):
                ot = outsb_pool.tile([P, FD], bf16, tag="ot",
                                     name=f"ot_{m}_{n}")
                nc.vector.tensor_scalar(
                    out=ot[:], in0=psum_tile[:], scalar1=sca[:],
                    scalar2=None, op0=Alu.mult,
                )
                base = n * FD
                if last:
                    # Final tile is on the critical tail: split the
                    # writeback across two queues.
                    nc.sync.dma_start(
                        out_d[m * P:(m + 1) * P, base:base + R], ot[:, 0:R]
                    )
                    nc.gpsimd.dma_start(
                        out_d[m * P:(m + 1) * P, base + R:base + FD],
                        ot[:, R:FD],
                    )
                    return
                eng = (nc.sync, nc.scalar, nc.gpsimd)[n % 3]
                eng.dma_start(
                    out_d[m * P:(m + 1) * P, base:base + FD], ot[:]
                )

            def emit_deg_scale(m, deg_ps):
                # scale = 64 / (16*deg1 + 1) == 1 / (0.25*deg1 + 0.015625)
                t1 = sc_pool.tile([P, 1], fp32, tag="t1", name=f"t1_{m}")
                nc.vector.tensor_scalar(
                    out=t1[:], in0=deg_ps[:], scalar1=0.25, scalar2=0.015625,
                    op0=Alu.mult, op1=Alu.add,
                )
                sca = sc_pool.tile([P, 1], fp32, tag="sca", name=f"sca_{m}")
                nc.vector.reciprocal(sca[:], t1[:])
                return sca

            # m = 0: jp-outer
            m = 0
            outs_ps = [psum_pool.tile([P, FD], fp32, tag="bank",
                                      name=f"outps0_{i}") for i in range(NT2)]
            deg_full = psum_pool.tile([P, FD], fp32, tag="bank", name="deg_0")
            deg_ps = deg_full[:, 0:1]
            for n in range(NT2):
                nc.tensor.matmul(
                    outs_ps[n][:], iq_t[:],
                    ar_t[:, m, n * FD:(n + 1) * FD],
                    start=True, stop=False, skip_group_check=True,
                )
            for t in range(KP):
                lhsT = mt_tiles[t][:, :, m * P:(m + 1) * P]
                for n in range(NT2):
                    nc.tensor.matmul(
                        outs_ps[n][:], lhsT,
                        a_tiles[t][:, :, n * FD:(n + 1) * FD],
                        start=False, stop=(t == KP - 1),
                        perf_mode=DR, skip_group_check=True,
                    )
                for i in range(2):
                    nc.tensor.matmul(
                        deg_ps[:], mt_tiles[t][:, i, m * P:(m + 1) * P],
                        ones_t[:],
                        start=(t == 0 and i == 0), stop=(t == KP - 1 and i == 1),
                    )
            sca = emit_deg_scale(m, deg_ps)
            for n in range(NT2):
                emit_epilogue(m, n, outs_ps[n], sca)

            # m = 1: n-outer, deg rides along with the n=0 bank
            m = 1
            deg_full = psum_pool.tile([P, FD], fp32, tag="bank", name="deg_1")
            deg_ps = deg_full[:, 0:1]
            sca = None
            for n in range(NT2):
                ops = psum_pool.tile([P, FD], fp32, tag="bank",
                                     name=f"outps1_{n}")
                nc.tensor.matmul(
                    ops[:], iq_t[:], ar_t[:, m, n * FD:(n + 1) * FD],
                    start=True, stop=False, skip_group_check=True,
                )
                for t in range(KP):
                    nc.tensor.matmul(
                        ops[:], mt_tiles[t][:, :, m * P:(m + 1) * P],
                        a_tiles[t][:, :, n * FD:(n + 1) * FD],
                        start=False, stop=(t == KP - 1),
                        perf_mode=DR, skip_group_check=True,
                    )
                    if n == 0:
                        for i in range(2):
                            nc.tensor.matmul(
                                deg_ps[:],
                                mt_tiles[t][:, i, m * P:(m + 1) * P],
                                ones_t[:],
                                start=(t == 0 and i == 0),
                                stop=(t == KP - 1 and i == 1),
                            )
                if n == 0:
                    sca = emit_deg_scale(m, deg_ps)
                emit_epilogue(m, n, ops, sca, last=(n == NT2 - 1))
    nc.compile()
    return nc


def _get_nc():
    if "nc" not in _CACHE:
        _CACHE["nc"] = _build_bass()
    return _CACHE["nc"]


def _make_in_maps(A_f32):
    import ml_dtypes

    f8 = ml_dtypes.float8_e4m3
    Ah = (A_f32 * 0.5).astype(f8)
    ATh = np.ascontiguousarray(Ah.T)
    # k-slab shuffle: a_shuf[p, k, :] = Ah[k*128 + p, :]
    a_shuf = np.ascontiguousarray(Ah.reshape(KT, P, N).transpose(1, 0, 2))

    ones = np.ones((P, 1), dtype=f8)
    iq = (0.0625 * np.eye(P, dtype=np.float32)).astype(f8)
    in_maps = []
    for c in range(NCORES):
        sl = slice(c * R, (c + 1) * R)
        art = np.ascontiguousarray(
            ATh[:, sl].reshape(KT, P, R).transpose(1, 0, 2)
        )
        ar = np.ascontiguousarray(
            Ah[sl, :].reshape(2, P, N).transpose(1, 0, 2)
        )
        in_maps.append({
            "a": a_shuf,
            "art": art,
            "ar": ar,
            "ones": ones,
            "iq": iq,
        })
    return in_maps


def kernel(A, w1a=None, w1b=None, w2a=None, **_unused):
    # w1a/w1b/w2a only enter the reference through a softmax over a
    # singleton axis (== 1.0), so the output does not depend on them.
    from concourse.bass_utils import run_bass_kernel_spmd

    A = np.asarray(A, dtype=np.float32)
    assert A.shape == (N, N), A.shape
    nc = _get_nc()
    in_maps = _make_in_maps(A)
    res = run_bass_kernel_spmd(nc, in_maps, core_ids=list(range(NCORES)))
    out = np.concatenate(
        [res.results[c]["out"] for c in range(NCORES)], axis=0
    )
    return out[None].astype(np.float32)


# revision 20
# speedup vs baseline: 1.0807x; 1.0639x over previous
"""Trainium2 Bass kernel for nn_GTN_72679436583060 (GTN message passing).

Math: with w-softmax over a singleton axis each GTConv is exactly 2*A, so

    out = 2 * rownorm(4*A@A + I) @ A
        = diag(64 / (16*rowsum(M1) + 1)) @ (M1@Ah + 0.0625*Ah_rows)

with Ah = A/2 (so M1 = Ah@Ah = (A@A)/4 ~ 128 fits fp8e4m3 range) and
rowsum/deg folded into a per-row reciprocal scale.

Sharding: row-wise over 8 cores, Ah replicated. Per core (rows R = 256):
  GEMM1 (transposed):  MT = Ah^T @ (Ah_rows^T)      (2048 x 256)
  deg:                 rowsum(M1) via ones-column matmuls on MT
  GEMM2:               P = M1 @ Ah + 0.0625*Ah_rows  (256 x 2048)
  epilogue:            out = P * (64 / (16*deg1 + 1)) per-row, bf16 out

All matmuls in fp8e4m3 with perf_mode=DoubleRow (K=256 per instruction,
two k-slabs per 3D access pattern [128, 2, f]), fp32 PSUM accumulation.
GEMM1 runs k-pair-outer so the PE tracks the streaming A DMA. Each HWDGE
queue sustains only ~150 GB/s, so the A stream is split slab-wise over
the sync+scalar queues while all small transfers (art/ar/iq/ones) ride
the gpsimd queue. Output is DMA'd as bf16 and upcast on the host.
"""

import numpy as np

N = 2048
P = 128
NCORES = 8
R = N // NCORES        # 256 rows per core
KT = N // P            # 16 partition tiles
KP = KT // 2           # 8 k-pair tiles (DoubleRow)
MT = R // P            # 2 row subtiles per core
FD = 512               # PSUM bank free dim (fp32)
NT2 = N // FD          # 4 GEMM2 n-tiles

_CACHE = {}


def _build_bass():
    from contextlib import ExitStack

    import concourse.bass as bass  # noqa: F401
    import concourse.mybir as mybir
    import concourse.tile as tile
    from concourse import bacc

    dt = mybir.dt
    fp32 = dt.float32
    bf16 = dt.bfloat16
    fp8 = dt.float8e4
    Alu = mybir.AluOpType
    DR = mybir.MatmulPerfMode.DoubleRow

    nc = bacc.Bacc(None, target_bir_lowering=False)
    # a/ar are k-slab shuffled on the host: [p, k, :] = X[k*128 + p, :].
    a_d = nc.dram_tensor("a", [P, KT, N], fp8, kind="ExternalInput")
    art_d = nc.dram_tensor("art", [P, KT, R], fp8, kind="ExternalInput")
    ar_d = nc.dram_tensor("ar", [P, 2, N], fp8, kind="ExternalInput")
    ones_d = nc.dram_tensor("ones", [P, 1], fp8, kind="ExternalInput")
    iq_d = nc.dram_tensor("iq", [P, P], fp8, kind="ExternalInput")
    out_d = nc.dram_tensor("out", [R, N], bf16, kind="ExternalOutput")

    with tile.TileContext(nc) as tc, ExitStack() as ctx:
        a_pool = ctx.enter_context(tc.tile_pool(name="a", bufs=KP))
        art_pool = ctx.enter_context(tc.tile_pool(name="art", bufs=KP))
        ar_pool = ctx.enter_context(tc.tile_pool(name="ar", bufs=1))
        mt_pool = ctx.enter_context(tc.tile_pool(name="mt", bufs=KP))
        const_pool = ctx.enter_context(tc.tile_pool(name="const", bufs=1))
        outsb_pool = ctx.enter_context(tc.tile_pool(name="outsb", bufs=4))
        sc_pool = ctx.enter_context(tc.tile_pool(name="sc", bufs=4))

        zeros_t = const_pool.tile([P, FD], bf16, tag="zeros")
        nc.vector.memset(zeros_t[:], 0.0)

        # Stream A k-slab pairs (and the matching ART pairs) in k order;
        # they stay resident: GEMM1 uses A slabs as lhsT, GEMM2 reuses
        # them as rhs. Slab 0 of each pair rides the sync queue behind its
        # art pair, slab 1 rides the scalar queue — a pair plus its art
        # lands every ~2us, tracking the GEMM1 k-sweep. The GEMM2-only
        # ar/ones/iq loads go last.
        art_tiles = [
            art_pool.tile([P, 2, R], fp8, tag="art", name=f"art_{t}")
            for t in range(KP)
        ]
        a_tiles = [
            a_pool.tile([P, 2, N], fp8, tag="a", name=f"a_{t}")
            for t in range(KP)
        ]
        for t in range(KP):
            nc.sync.dma_start(art_tiles[t][:], art_d[:, 2 * t:2 * t + 2, :])
            nc.sync.dma_start(a_tiles[t][:, 0, :], a_d[:, 2 * t, :])
            nc.scalar.dma_start(a_tiles[t][:, 1, :], a_d[:, 2 * t + 1, :])
        ar_t = ar_pool.tile([P, 2, N], fp8, tag="ar")
        nc.scalar.dma_start(ar_t[:], ar_d[:, :, :])
        ones_t = const_pool.tile([P, 1], fp8, tag="ones")
        nc.sync.dma_start(ones_t[:], ones_d[:, :])
        iq_t = const_pool.tile([P, P], fp8, tag="iq")
        nc.sync.dma_start(iq_t[:], iq_d[:, :])

        # ---- GEMM1: MT[j, r] = sum_k Ah[k, j] * Ah_rows[r, k] ----
        # k-pair outer (DoubleRow contracts 256 rows per matmul). Two
        # j-groups share each PSUM bank: the even j's first matmul carries
        # start=True (resets has_written for the whole bank), the odd j's
        # first matmul then overwrites its all-unwritten half; later
        # matmuls accumulate. The PE executes its queue in program order,
        # so the start=True matmul always lands first. Warmup matmuls
        # during the initial DMA window ramp the PE HAM clock; they write
        # bank 0, and the t=0 j-sweep visits j=0,1 last so their WAW
        # ordering after the warmup chain never stalls the sweep (bank 0's
        # real start=True matmul discards the warmup garbage).
        with tc.tile_pool(name="psum", bufs=8, space="PSUM") as psum_pool:
            pairs = []
            for b in range(KP):
                ps = psum_pool.tile([P, FD], fp32, tag="bank", name=f"pair_{b}")
                pairs.append(ps)
            for w in range(8):
                nc.tensor.matmul(
                    pairs[0][:], zeros_t[:, 0:P], zeros_t[:],
                    start=(w == 0), stop=False, skip_group_check=True,
                )
            for t in range(KP):
                jorder = list(range(2, KT)) + [0, 1] if t == 0 else range(KT)
                for j in jorder:
                    half = (j % 2) * R
                    nc.tensor.matmul(
                        pairs[j // 2][:, half:half + R],
                        a_tiles[t][:, :, j * P:(j + 1) * P],
                        art_tiles[t][:],
                        start=(t == 0 and j % 2 == 0), stop=(t == KP - 1),
                        perf_mode=DR, skip_group_check=True,
                    )
            # PSUM -> SBUF fp8 quantization of MT (values ~128 < 240 max)
            mt_tiles = [None] * KP
            for j in range(KT):
                half = (j % 2) * R
                if j % 2 == 0:
                    mt_tiles[j // 2] = mt_pool.tile(
                        [P, 2, R], fp8, tag="mt", name=f"mt_{j // 2}"
                    )
                nc.vector.tensor_copy(
                    mt_tiles[j // 2][:, j % 2, :],
                    pairs[j // 2][:, half:half + R],
                )

            # ---- GEMM2 + deg + epilogue ----
            # The 0.0625*I seed matmul doubles as each bank's accumulation
            # starter (start=True clears the bank and seeds 0.0625*Ah_rows).
            # m=0 runs jp-outer (tracks the mt copies); m=1 runs n-outer so
            # its banks finish staggered and the epilogues pipeline with PE.
            def emit_epilogue(m, n, psum_tile, sca, last=False):
                ot = outsb_pool.tile([P, FD], bf16, tag="ot",
                                     name=f"ot_{m}_{n}")
                nc.vector.tensor_scalar(
                    out=ot[:], in0=psum_tile[:], scalar1=sca[:],
                    scalar2=None, op0=Alu.mult,
                )
                base = n * FD
                if last:
                    # Final tile is on the critical tail: split the
                    # writeback across two queues.
                    nc.sync.dma_start(
                        out_d[m * P:(m + 1) * P, base:base + R], ot[:, 0:R]
                    )
                    nc.scalar.dma_start(
                        out_d[m * P:(m + 1) * P, base + R:base + FD],
                        ot[:, R:FD],
                    )
                    return
                eng = nc.sync if n % 2 == 0 else nc.scalar
                eng.dma_start(
                    out_d[m * P:(m + 1) * P, base:base + FD], ot[:]
                )

            def emit_deg_scale(m, deg_ps):
                # scale = 64 / (16*deg1 + 1) == 1 / (0.25*deg1 + 0.015625)
                t1 = sc_pool.tile([P, 1], fp32, tag="t1", name=f"t1_{m}")
                nc.vector.tensor_scalar(
                    out=t1[:], in0=deg_ps[:], scalar1=0.25, scalar2=0.015625,
                    op0=Alu.mult, op1=Alu.add,
                )
                sca = sc_pool.tile([P, 1], fp32, tag="sca", name=f"sca_{m}")
                nc.vector.reciprocal(sca[:], t1[:])
                return sca

            # m = 0: jp-outer
            m = 0
            outs_ps = [psum_pool.tile([P, FD], fp32, tag="bank",
                                      name=f"outps0_{i}") for i in range(NT2)]
            deg_full = psum_pool.tile([P, FD], fp32, tag="bank", name="deg_0")
            deg_ps = deg_full[:, 0:1]
            for n in range(NT2):
                nc.tensor.matmul(
                    outs_ps[n][:], iq_t[:],
                    ar_t[:, m, n * FD:(n + 1) * FD],
                    start=True, stop=False, skip_group_check=True,
                )
            for t in range(KP):
                lhsT = mt_tiles[t][:, :, m * P:(m + 1) * P]
                for n in range(NT2):
                    nc.tensor.matmul(
                        outs_ps[n][:], lhsT,
                        a_tiles[t][:, :, n * FD:(n + 1) * FD],
                        start=False, stop=(t == KP - 1),
                        perf_mode=DR, skip_group_check=True,
                    )
                for i in range(2):
                    nc.tensor.matmul(
                        deg_ps[:], mt_tiles[t][:, i, m * P:(m + 1) * P],
                        ones_t[:],
                        start=(t == 0 and i == 0), stop=(t == KP - 1 and i == 1),
                    )
            sca = emit_deg_scale(m, deg_ps)
            for n in range(NT2):
                emit_epilogue(m, n, outs_ps[n], sca)

            # m = 1: n-outer, deg rides along with the n=0 bank
            m = 1
            deg_full = psum_pool.tile([P, FD], fp32, tag="bank", name="deg_1")
            deg_ps = deg_full[:, 0:1]
            sca = None
            for n in range(NT2):
                ops = psum_pool.tile([P, FD], fp32, tag="bank",
                                     name=f"outps1_{n}")
                nc.tensor.matmul(
                    ops[:], iq_t[:], ar_t[:, m, n * FD:(n + 1) * FD],
                    start=True, stop=False, skip_group_check=True,
                )
                for t in range(KP):
                    nc.tensor.matmul(
                        ops[:], mt_tiles[t][:, :, m * P:(m + 1) * P],
                        a_tiles[t][:, :, n * FD:(n + 1) * FD],
                        start=False, stop=(t == KP - 1),
                        perf_mode=DR, skip_group_check=True,
                    )
                    if n == 0:
                        for i in range(2):
                            nc.tensor.matmul(
                                deg_ps[:],
                                mt_tiles[t][:, i, m * P:(m + 1) * P],
                                ones_t[:],
                                start=(t == 0 and i == 0),
                                stop=(t == KP - 1 and i == 1),
                            )
                if n == 0:
                    sca = emit_deg_scale(m, deg_ps)
                emit_epilogue(m, n, ops, sca, last=(n == NT2 - 1))
    nc.compile()
    return nc


def _get_nc():
    if "nc" not in _CACHE:
        _CACHE["nc"] = _build_bass()
    return _CACHE["nc"]


def _make_in_maps(A_f32):
    import ml_dtypes

    f8 = ml_dtypes.float8_e4m3
    Ah = (A_f32 * 0.5).astype(f8)
    ATh = np.ascontiguousarray(Ah.T)
    # k-slab shuffle: a_shuf[p, k, :] = Ah[k*128 + p, :]
    a_shuf = np.ascontiguousarray(Ah.reshape(KT, P, N).transpose(1, 0, 2))

    ones = np.ones((P, 1), dtype=f8)
    iq = (0.0625 * np.eye(P, dtype=np.float32)).astype(f8)
    in_maps = []
    for c in range(NCORES):
        sl = slice(c * R, (c + 1) * R)
        art = np.ascontiguousarray(
            ATh[:, sl].reshape(KT, P, R).transpose(1, 0, 2)
        )
        ar = np.ascontiguousarray(
            Ah[sl, :].reshape(2, P, N).transpose(1, 0, 2)
        )
        in_maps.append({
            "a": a_shuf,
            "art": art,
            "ar": ar,
            "ones": ones,
            "iq": iq,
        })
    return in_maps


def kernel(A, w1a=None, w1b=None, w2a=None, **_unused):
    # w1a/w1b/w2a only enter the reference through a softmax over a
    # singleton axis (== 1.0), so the output does not depend on them.
    from concourse.bass_utils import run_bass_kernel_spmd

    A = np.asarray(A, dtype=np.float32)
    assert A.shape == (N, N), A.shape
    nc = _get_nc()
    in_maps = _make_in_maps(A)
    res = run_bass_kernel_spmd(nc, in_maps, core_ids=list(range(NCORES)))
    out = np.concatenate(
        [res.results[c]["out"] for c in range(NCORES)], axis=0
    )
    return out[None].astype(np.float32)
